# revision 2
# baseline (speedup 1.0000x reference)
"""Bass/Tile TRN2 kernel for LowRankMixtureCrossNet (B=16384, N=1024, L=3, E=8, R=64).

Strategy:
- Data-parallel: batch sharded 8 ways (2048 tokens/core), weights replicated.
- On-chip layout is feature-major (x^T): SBUF tiles [128 feat, T=512 tokens].
  Host pre-transposes x and pre-packs the weights.
- All matmuls in fp32r (TF32-like, 1 cycle/row at free>=256, ~2^-11 rel err).
- Per layer, per token tile:
    logits[e,t]  = sum_n gate_w[e,n] x[n,t]           (8 chunk matmuls, M=8)
    gates        = softmax over e: exp (ACT), partition-reduce + broadcast
                   (GPSIMD), reciprocal+mult (DVE)
    v            = 4 expert-pair matmuls x 8 K-chunks, M=128 (2 experts x R=64)
    rvg          = relu(v) * gates  (gates folded in early:
                   g*U@relu(C@(g*relu(v))) == g*u since g>0 commutes via relu)
    cg           = relu(Cblk @ rvg)        (block-diag 2-expert C matmuls)
    w            = Uall^T.T @ cg           (8 n-chunks x 4 K-pair matmuls)
    xnew[n,t]    = (w[n,t] + bias[n]) * x0[n,t] + x[n,t]
      (softmax makes sum_e g = 1, so bias needs no gate weighting)
"""
import numpy as np
from contextlib import ExitStack

import concourse.bass as bass
import concourse.tile as tile
from concourse import bacc, mybir
from concourse.bass_utils import run_bass_kernel_spmd

B, N, L, E, R = 16384, 1024, 3, 8, 64
NCORES = 8
BC = B // NCORES      # tokens per core
T = 512               # token tile (matmul free dim)
NT = BC // T          # token tiles per core
NCH = N // 128        # feature chunks
NP = E // 2           # expert pairs
ER = E * R            # 512

f32 = mybir.dt.float32
f32r = mybir.dt.float32r
AFT = mybir.ActivationFunctionType
ALU = mybir.AluOpType


def build(niter: int = 1):
    nc = bacc.Bacc(trn_type="TRN2", debug=False, num_devices=NCORES)

    xT_d = nc.dram_tensor("xT", [N, BC], f32r, kind="ExternalInput")
    vt_d = nc.dram_tensor("VT", [L, N, ER], f32r, kind="ExternalInput")
    ut_d = nc.dram_tensor("UT", [L, ER, N], f32r, kind="ExternalInput")
    cb_d = nc.dram_tensor("CB", [L, 128, NP * 128], f32r, kind="ExternalInput")
    gt_d = nc.dram_tensor("GT", [N, E], f32r, kind="ExternalInput")
    sel_d = nc.dram_tensor("SEL", [E, NP * 128], f32r, kind="ExternalInput")
    bs_d = nc.dram_tensor("BS", [128, L * NCH], f32, kind="ExternalInput")
    y_d = nc.dram_tensor("y", [N, BC], f32, kind="ExternalOutput")

    with tile.TileContext(nc) as tc, ExitStack() as ctx:
        wp = ctx.enter_context(tc.tile_pool(name="wp", bufs=1))
        xp = ctx.enter_context(tc.tile_pool(name="xp", bufs=2))
        xc = ctx.enter_context(tc.tile_pool(name="xc", bufs=2))
        wk = ctx.enter_context(tc.tile_pool(name="wk", bufs=2))
        g8 = ctx.enter_context(tc.tile_pool(name="g8", bufs=1))
        pv = ctx.enter_context(tc.tile_pool(name="pv", bufs=2, space="PSUM"))
        pcg = ctx.enter_context(tc.tile_pool(name="pcg", bufs=3, space="PSUM"))
        pw = ctx.enter_context(tc.tile_pool(name="pw", bufs=2, space="PSUM"))
        pg = ctx.enter_context(tc.tile_pool(name="pg", bufs=1, space="PSUM"))

        # ---- persistent weights ----
        vt, ut, cbt, gt = {}, {}, {}, {}
        for l in range(L):
            for c in range(NCH):
                t = wp.tile([128, ER], f32r, tag=f"vt{l}_{c}", name=f"vt{l}_{c}")
                nc.sync.dma_start(t[:], vt_d[l, c * 128:(c + 1) * 128, :])
                vt[l, c] = t
            for k in range(NP):
                t = wp.tile([128, N], f32r, tag=f"ut{l}_{k}", name=f"ut{l}_{k}")
                nc.sync.dma_start(t[:], ut_d[l, k * 128:(k + 1) * 128, :])
                ut[l, k] = t
            t = wp.tile([128, NP * 128], f32r, tag=f"cb{l}", name=f"cb{l}")
            nc.sync.dma_start(t[:], cb_d[l, :, :])
            cbt[l] = t
        for c in range(NCH):
            t = wp.tile([128, E], f32r, tag=f"gt{c}", name=f"gt{c}")
            nc.sync.dma_start(t[:], gt_d[c * 128:(c + 1) * 128, :])
            gt[c] = t
        selt = wp.tile([E, NP * 128], f32r, tag="sel", name="selt")
        nc.sync.dma_start(selt[:], sel_d[:, :])
        bst = wp.tile([128, L * NCH], f32, tag="bs", name="bst")
        nc.sync.dma_start(bst[:], bs_d[:, :])

        uid = [0]

        def token_tile(t):
            uid[0] += 1
            u = uid[0]
            x0 = [xp.tile([128, T], f32r, tag=f"x0_{c}", name=f"x0_{u}_{c}")
                  for c in range(NCH)]
            for c in range(NCH):
                nc.sync.dma_start(x0[c][:], xT_d[c * 128:(c + 1) * 128, t * T:(t + 1) * T])
            xcur = [xc.tile([128, T], f32r, tag=f"xc_{c}", name=f"xc_{u}_{c}")
                    for c in range(NCH)]
            for l in range(L):
                xin = x0 if l == 0 else xcur
                # ---- gates ----
                lg = pg.tile([E, T], f32, tag="lg", name=f"lg_{u}_{l}")
                for c in range(NCH):
                    nc.tensor.matmul(lg[:], lhsT=gt[c][:], rhs=xin[c][:],
                                     start=(c == 0), stop=(c == NCH - 1))
                eh = g8.tile([E, T], f32, tag="eh", name=f"eh_{u}_{l}")
                nc.scalar.activation(eh[:], lg[:], AFT.Exp)
                S = g8.tile([1, T], f32, tag="S", name=f"S_{u}_{l}")
                nc.gpsimd.tensor_reduce(S[:], eh[:], axis=mybir.AxisListType.C,
                                        op=ALU.add)
                r1 = g8.tile([1, T], f32, tag="r1", name=f"r1_{u}_{l}")
                nc.vector.reciprocal(r1[:], S[:])
                r8 = g8.tile([E, T], f32, tag="r8", name=f"r8_{u}_{l}")
                nc.gpsimd.partition_broadcast(r8[:], r1[:])
                gn = g8.tile([E, T], f32r, tag="gn", name=f"gn_{u}_{l}")
                nc.vector.tensor_tensor(gn[:], eh[:], r8[:], op=ALU.mult)

                # ---- expert low-rank chain, per expert pair ----
                vps = {}
                for p in range(NP):
                    vp = pv.tile([128, T], f32, tag="v", name=f"v_{u}_{l}_{p}")
                    for c in range(NCH):
                        nc.tensor.matmul(vp[:], lhsT=vt[l, c][:, p * 128:(p + 1) * 128],
                                         rhs=xin[c][:],
                                         start=(c == 0), stop=(c == NCH - 1))
                    vps[p] = vp
                cg = {}
                for p in range(NP):
                    g2 = pcg.tile([128, T], f32, tag="cg2", name=f"g2_{u}_{l}_{p}")
                    nc.tensor.matmul(g2[:], lhsT=selt[:, p * 128:(p + 1) * 128],
                                     rhs=gn[:], start=True, stop=True)
                    rv = wk.tile([128, T], f32, tag="rv", name=f"rv_{u}_{l}_{p}")
                    nc.scalar.activation(rv[:], vps[p][:], AFT.Relu)
                    rvg = wk.tile([128, T], f32r, tag="rvg", name=f"rvg_{u}_{l}_{p}")
                    nc.vector.tensor_tensor(rvg[:], rv[:], g2[:], op=ALU.mult)
                    cp = pcg.tile([128, T], f32, tag="cg2", name=f"c_{u}_{l}_{p}")
                    nc.tensor.matmul(cp[:], lhsT=cbt[l][:, p * 128:(p + 1) * 128],
                                     rhs=rvg[:], start=True, stop=True)
                    cgp = wk.tile([128, T], f32r, tag=f"cg{p}", name=f"cg_{u}_{l}_{p}", bufs=1)
                    nc.scalar.activation(cgp[:], cp[:], AFT.Relu)
                    cg[p] = cgp

                # ---- u-projection + residual update ----
                for m in range(NCH):
                    wm = pw.tile([128, T], f32, tag="w", name=f"w_{u}_{l}_{m}")
                    for k in range(NP):
                        nc.tensor.matmul(wm[:], lhsT=ut[l, k][:, m * 128:(m + 1) * 128],
                                         rhs=cg[k][:],
                                         start=(k == 0), stop=(k == NP - 1))
                    t2 = wk.tile([128, T], f32, tag="t2", name=f"t2_{u}_{l}_{m}")
                    nc.vector.scalar_tensor_tensor(
                        t2[:], wm[:], bst[:, l * NCH + m:l * NCH + m + 1],
                        x0[m][:].bitcast(f32), op0=ALU.add, op1=ALU.mult)
                    nc.vector.tensor_tensor(xcur[m][:], t2[:], xin[m][:].bitcast(f32),
                                            op=ALU.add)
            for c in range(NCH):
                nc.sync.dma_start(y_d[c * 128:(c + 1) * 128, t * T:(t + 1) * T],
                                  xcur[c][:].bitcast(f32))

        if niter == 1:
            for t in range(NT):
                token_tile(t)
        else:
            with tc.For_i(0, niter, 1) as _:
                for t in range(NT):
                    token_tile(t)

    nc.compile()
    return nc


def pack_inputs(x, U, V, C, bias, gate_w):
    """Host-side packing into the DRAM layouts the kernel expects."""
    x = np.asarray(x, dtype=np.float32)
    U = np.asarray(U, dtype=np.float32)
    V = np.asarray(V, dtype=np.float32)
    C = np.asarray(C, dtype=np.float32)
    bias = np.asarray(bias, dtype=np.float32)
    gate_w = np.asarray(gate_w, dtype=np.float32)

    xT = np.ascontiguousarray(x.T)                          # [N, B]
    VT = np.ascontiguousarray(V.transpose(0, 3, 1, 2).reshape(L, N, ER))
    UT = np.ascontiguousarray(U.transpose(0, 1, 3, 2).reshape(L, ER, N))
    CB = np.zeros((L, 128, NP * 128), np.float32)
    for l in range(L):
        for p in range(NP):
            CB[l, 0:64, p * 128:p * 128 + 64] = C[l, 2 * p].T
            CB[l, 64:128, p * 128 + 64:p * 128 + 128] = C[l, 2 * p + 1].T
    GT = np.ascontiguousarray(gate_w.T)                     # [N, E]
    SEL = np.zeros((E, NP * 128), np.float32)
    for p in range(NP):
        SEL[2 * p, p * 128:p * 128 + 64] = 1.0
        SEL[2 * p + 1, p * 128 + 64:p * 128 + 128] = 1.0
    BS = np.zeros((128, L * NCH), np.float32)
    for l in range(L):
        for m in range(NCH):
            BS[:, l * NCH + m] = bias[l, m * 128:(m + 1) * 128]

    shared = {"VT": VT, "UT": UT, "CB": CB, "GT": GT, "SEL": SEL, "BS": BS}
    in_maps = []
    for i in range(NCORES):
        m = dict(shared)
        m["xT"] = np.ascontiguousarray(xT[:, i * BC:(i + 1) * BC])
        in_maps.append(m)
    return in_maps


def run(nc, in_maps):
    res = run_bass_kernel_spmd(nc, in_maps, core_ids=list(range(NCORES)))
    yT = np.empty((N, B), np.float32)
    for i in range(NCORES):
        yT[:, i * BC:(i + 1) * BC] = res.results[i]["y"]
    return np.ascontiguousarray(yT.T)


def kernel(x, U, V, C, bias, gate_w):
    nc = build(niter=1)
    in_maps = pack_inputs(x, U, V, C, bias, gate_w)
    return run(nc, in_maps)


# revision 20
# speedup vs baseline: 2.8685x; 2.8685x over previous
"""Bass/Tile TRN2 kernel for LowRankMixtureCrossNet (B=16384, N=1024, L=3, E=8, R=64).

Strategy:
- Data-parallel: batch sharded 8 ways (2048 tokens/core), weights replicated.
- On-chip layout is feature-major (x^T): SBUF tiles [128 feat, T=512 tokens].
  Host pre-transposes x and pre-packs the weights.
- All matmuls in fp32r (TF32-like, 1 cycle/row at free>=256, ~2^-11 rel err).
- Per layer, per token tile:
    logits[e,t]  = sum_n gate_w[e,n] x[n,t]           (8 chunk matmuls, M=8)
    gates        = softmax over e: exp (ACT), partition-reduce + broadcast
                   (GPSIMD), reciprocal+mult (DVE)
    v            = 4 expert-pair matmuls x 8 K-chunks, M=128 (2 experts x R=64)
    rvg          = relu(v) * gates  (gates folded in early:
                   g*U@relu(C@(g*relu(v))) == g*u since g>0 commutes via relu)
    cg           = relu(Cblk @ rvg)        (block-diag 2-expert C matmuls)
    w            = Uall^T.T @ cg           (8 n-chunks x 4 K-pair matmuls)
    xnew[n,t]    = (w[n,t] + bias[n]) * x0[n,t] + x[n,t]
      (softmax makes sum_e g = 1, so bias needs no gate weighting)
"""
import numpy as np
from contextlib import ExitStack

import concourse.bass as bass
import concourse.tile as tile
from concourse import bacc, mybir
from concourse.bass_utils import run_bass_kernel_spmd

B, N, L, E, R = 16384, 1024, 3, 8, 64
NCORES = 8
BC = B // NCORES      # tokens per core
T = 512               # token tile (matmul free dim)
NT = BC // T          # token tiles per core
NCH = N // 128        # feature chunks
NP = E // 2           # expert pairs
ER = E * R            # 512

f32 = mybir.dt.float32
f32r = mybir.dt.float32r
bf16 = mybir.dt.bfloat16
AFT = mybir.ActivationFunctionType
ALU = mybir.AluOpType


def build(niter: int = 1, dma_in_loop=True, elemwise=True, matmuls=True, mmdt="f32r", psum=(2, 4, 2)):
    MDT = {"f32r": f32r, "bf16": bf16}[mmdt]
    nc = bacc.Bacc(trn_type="TRN2", debug=False, num_devices=NCORES)

    xT_d = nc.dram_tensor("xT", [N, BC], MDT, kind="ExternalInput")
    vt_d = nc.dram_tensor("VT", [L, N, ER], MDT, kind="ExternalInput")
    ut_d = nc.dram_tensor("UT", [L, ER, N], MDT, kind="ExternalInput")
    cb_d = nc.dram_tensor("CB", [L, 128, NP * 128], MDT, kind="ExternalInput")
    gt_d = nc.dram_tensor("GT", [N, E], MDT, kind="ExternalInput")
    sel_d = nc.dram_tensor("SEL", [E, NP * 128], MDT, kind="ExternalInput")
    bs_d = nc.dram_tensor("BS", [128, L * NCH], f32, kind="ExternalInput")
    on8_d = nc.dram_tensor("ON8", [E, 1], MDT, kind="ExternalInput")
    on1_d = nc.dram_tensor("ON1", [1, E], MDT, kind="ExternalInput")
    y_d = nc.dram_tensor("y", [N, BC], f32, kind="ExternalOutput")

    with tile.TileContext(nc) as tc, ExitStack() as ctx:
        wp = ctx.enter_context(tc.tile_pool(name="wp", bufs=1))
        xp = ctx.enter_context(tc.tile_pool(name="xp", bufs=2))
        xc = ctx.enter_context(tc.tile_pool(name="xc", bufs=2))
        wk = ctx.enter_context(tc.tile_pool(name="wk", bufs=3))
        g8 = ctx.enter_context(tc.tile_pool(name="g8", bufs=1))
        pv = ctx.enter_context(tc.tile_pool(name="pv", bufs=psum[0], space="PSUM"))
        pcg = ctx.enter_context(tc.tile_pool(name="pcg", bufs=psum[1], space="PSUM"))
        pw = ctx.enter_context(tc.tile_pool(name="pw", bufs=psum[2], space="PSUM"))

        # ---- persistent weights ----
        vt, ut, cbt, gt = {}, {}, {}, {}
        vtl, utl = {}, {}

        def load_layer_weights(l, eng):
            tv = wp.tile([128, NCH * ER], MDT, tag=f"vt{l}", name=f"vt{l}")
            eng.dma_start(tv[:].rearrange("p (c e) -> p c e", c=NCH),
                          vt_d[l].rearrange("(c p) e -> p c e", p=128))
            vtl[l] = tv
            for c in range(NCH):
                vt[l, c] = tv[:, c * ER:(c + 1) * ER]
            tu = wp.tile([128, NP * N], MDT, tag=f"ut{l}", name=f"ut{l}")
            eng.dma_start(tu[:].rearrange("p (k n) -> p k n", k=NP),
                          ut_d[l].rearrange("(k p) n -> p k n", p=128))
            utl[l] = tu
            for k in range(NP):
                ut[l, k] = tu[:, k * N:(k + 1) * N]
            t = wp.tile([128, NP * 128], MDT, tag=f"cb{l}", name=f"cb{l}")
            eng.dma_start(t[:], cb_d[l, :, :])
            cbt[l] = t

        # tiny operands + layer-0 V on the sync queue (critical path to the
        # first matmuls); the bulk (U0 + layers 1-2) on the scalar queue,
        # which is idle during preload.
        gtt = wp.tile([128, NCH * E], MDT, tag="gt", name="gtt")
        nc.sync.dma_start(gtt[:].rearrange("p (c e) -> p c e", c=NCH),
                          gt_d[:, :].rearrange("(c p) e -> p c e", p=128))
        for c in range(NCH):
            gt[c] = gtt[:, c * E:(c + 1) * E]
        selt = wp.tile([E, NP * 128], MDT, tag="sel", name="selt")
        nc.sync.dma_start(selt[:], sel_d[:, :])
        bst = wp.tile([128, L * NCH], f32, tag="bs", name="bst")
        nc.sync.dma_start(bst[:], bs_d[:, :])
        on8 = wp.tile([E, 1], MDT, tag="on8", name="on8")
        nc.sync.dma_start(on8[:], on8_d[:, :])
        on1 = wp.tile([1, E], MDT, tag="on1", name="on1")
        nc.sync.dma_start(on1[:], on1_d[:, :])
        tv = wp.tile([128, NCH * ER], MDT, tag="vt0", name="vt0")
        nc.sync.dma_start(tv[:].rearrange("p (c e) -> p c e", c=NCH),
                          vt_d[0].rearrange("(c p) e -> p c e", p=128))
        vtl[0] = tv
        for c in range(NCH):
            vt[0, c] = tv[:, c * ER:(c + 1) * ER]
        tu = wp.tile([128, NP * N], MDT, tag="ut0", name="ut0")
        nc.scalar.dma_start(tu[:].rearrange("p (k n) -> p k n", k=NP),
                            ut_d[0].rearrange("(k p) n -> p k n", p=128))
        utl[0] = tu
        for k in range(NP):
            ut[0, k] = tu[:, k * N:(k + 1) * N]
        t0cb = wp.tile([128, NP * 128], MDT, tag="cb0", name="cb0")
        nc.scalar.dma_start(t0cb[:], cb_d[0, :, :])
        cbt[0] = t0cb
        for l in range(1, L):
            load_layer_weights(l, nc.scalar)

        uid = [0]
        x0_static = {}

        def load_x0(t, u):
            x0 = [xp.tile([128, T], MDT, tag=f"x0_{c}", name=f"x0_{u}_{c}")
                  for c in range(NCH)]
            for c in range(NCH):
                nc.sync.dma_start(x0[c][:], xT_d[c * 128:(c + 1) * 128, t * T:(t + 1) * T])
            return [x0[c][:] for c in range(NCH)]

        def token_tile(t):
            uid[0] += 1
            u = uid[0]
            if dma_in_loop:
                x0 = load_x0(t, u)
            else:
                x0 = x0_static[t]
            xcurt = [xc.tile([128, T], MDT, tag=f"xc_{c}", name=f"xc_{u}_{c}")
                     for c in range(NCH)]
            xcur = [xcurt[c][:] for c in range(NCH)]
            for l in range(L):
                xin = x0 if l == 0 else xcur
                # ---- gate logits (PE) + exp (ACT) ----
                lg = pw.tile([E, T], f32, tag="w", name=f"lg_{u}_{l}")
                for c in range(NCH):
                    nc.tensor.matmul(lg[:], lhsT=gt[c][:], rhs=xin[c],
                                     start=(c == 0), stop=(c == NCH - 1))
                if elemwise:
                    eh = g8.tile([E, T], f32r, tag="eh", name=f"eh_{u}_{l}")
                    nc.scalar.activation(eh[:], lg[:], AFT.Exp)

                # ---- v matmuls (PE) with inline relu (ACT) ----
                rvs = {}
                for p in range(NP):
                    vp = pv.tile([128, T], f32, tag="v", name=f"v_{u}_{l}_{p}")
                    for c in range(NCH):
                        nc.tensor.matmul(vp[:], lhsT=vt[l, c][:, p * 128:(p + 1) * 128],
                                         rhs=xin[c],
                                         start=(c == 0), stop=(c == NCH - 1))
                    if elemwise:
                        rv = wk.tile([128, T], f32, tag="rv", name=f"rv_{u}_{l}_{p}", bufs=4)
                        nc.scalar.activation(rv[:], vp[:], AFT.Relu)
                        rvs[p] = rv

                # ---- softmax normalization (PE sum + DVE recip + PE bcast) ----
                if elemwise:
                    S = pw.tile([1, T], f32, tag="w", name=f"S_{u}_{l}")
                    nc.tensor.matmul(S[:], lhsT=on8[:], rhs=eh[:], start=True, stop=True)
                    r1 = g8.tile([1, T], f32r, tag="r1", name=f"r1_{u}_{l}")
                    with nc.allow_low_precision(reason="softmax recip to f32r"):
                        nc.vector.reciprocal(r1[:], S[:])
                    r8 = pw.tile([E, T], f32, tag="w", name=f"r8_{u}_{l}")
                    nc.tensor.matmul(r8[:], lhsT=on1[:], rhs=r1[:], start=True, stop=True)
                    gn = g8.tile([E, T], f32r, tag="gn", name=f"gn_{u}_{l}")
                    nc.vector.tensor_tensor(gn[:], eh[:].bitcast(f32), r8[:], op=ALU.mult)
                else:
                    gn = selt

                # ---- gate broadcast (PE), gated relu(v) (DVE), C matmuls (PE) ----
                g2s = {}
                for p in range(NP):
                    g2 = pcg.tile([128, T], f32, tag="cg2", name=f"g2_{u}_{l}_{p}")
                    nc.tensor.matmul(g2[:], lhsT=selt[:, p * 128:(p + 1) * 128],
                                     rhs=gn[:, 0:T], start=True, stop=True)
                    g2s[p] = g2
                rvgs = {}
                for p in range(NP):
                    if elemwise:
                        rvg = wk.tile([128, T], f32r, tag="rvg", name=f"rvg_{u}_{l}_{p}", bufs=4)
                        nc.vector.tensor_tensor(rvg[:], rvs[p][:], g2s[p][:], op=ALU.mult)
                        rvgs[p] = rvg[:]
                    else:
                        rvgs[p] = x0[p]
                cg = {}
                cps = {}
                for p in range(NP):
                    cp = pcg.tile([128, T], f32, tag="cg2", name=f"c_{u}_{l}_{p}")
                    nc.tensor.matmul(cp[:], lhsT=cbt[l][:, p * 128:(p + 1) * 128],
                                     rhs=rvgs[p], start=True, stop=True)
                    cps[p] = cp
                for p in range(NP):
                    if elemwise:
                        cgp = wk.tile([128, T], f32r, tag=f"cg{p}", name=f"cg_{u}_{l}_{p}", bufs=1)
                        nc.scalar.activation(cgp[:], cps[p][:], AFT.Relu)
                        cg[p] = cgp[:]
                    else:
                        cg[p] = x0[p]

                # ---- u-projection + residual update ----
                for m in range(NCH):
                    wm = pw.tile([128, T], f32, tag="w", name=f"w_{u}_{l}_{m}")
                    for k in range(NP):
                        nc.tensor.matmul(wm[:], lhsT=ut[l, k][:, m * 128:(m + 1) * 128],
                                         rhs=cg[k],
                                         start=(k == 0), stop=(k == NP - 1))
                    if elemwise:
                        t2 = wk.tile([128, T], f32, tag="t2", name=f"t2_{u}_{l}_{m}")
                        nc.vector.scalar_tensor_tensor(
                            t2[:], wm[:], bst[:, l * NCH + m:l * NCH + m + 1],
                            x0[m].bitcast(f32), op0=ALU.add, op1=ALU.mult)
                        nc.vector.tensor_tensor(xcur[m], t2[:], xin[m].bitcast(f32),
                                                op=ALU.add)
                    else:
                        nc.vector.tensor_copy(xcur[m], x0[m])
            if dma_in_loop and mmdt == "f32r":
                for c in range(NCH):
                    nc.sync.dma_start(y_d[c * 128:(c + 1) * 128, t * T:(t + 1) * T],
                                      xcur[c].bitcast(f32))

        if not dma_in_loop:
            shared_x0 = load_x0(0, 1000)
            for t in range(NT):
                x0_static[t] = shared_x0
        if niter == 1:
            for t in range(NT):
                token_tile(t)
        else:
            with tc.For_i(0, niter, 1) as _:
                for t in range(NT):
                    token_tile(t)
        if not dma_in_loop and mmdt == "f32r":
            for c in range(NCH):
                nc.sync.dma_start(y_d[c * 128:(c + 1) * 128, 0:T],
                                  x0_static[0][c].bitcast(f32))

    nc.compile()
    return nc


def pack_inputs(x, U, V, C, bias, gate_w, mmdt="f32r"):
    """Host-side packing into the DRAM layouts the kernel expects."""
    x = np.asarray(x, dtype=np.float32)
    U = np.asarray(U, dtype=np.float32)
    V = np.asarray(V, dtype=np.float32)
    C = np.asarray(C, dtype=np.float32)
    bias = np.asarray(bias, dtype=np.float32)
    gate_w = np.asarray(gate_w, dtype=np.float32)

    xT = np.ascontiguousarray(x.T)                          # [N, B]
    VT = np.ascontiguousarray(V.transpose(0, 3, 1, 2).reshape(L, N, ER))
    UT = np.ascontiguousarray(U.transpose(0, 1, 3, 2).reshape(L, ER, N))
    CB = np.zeros((L, 128, NP * 128), np.float32)
    for l in range(L):
        for p in range(NP):
            CB[l, 0:64, p * 128:p * 128 + 64] = C[l, 2 * p].T
            CB[l, 64:128, p * 128 + 64:p * 128 + 128] = C[l, 2 * p + 1].T
    GT = np.ascontiguousarray(gate_w.T)                     # [N, E]
    SEL = np.zeros((E, NP * 128), np.float32)
    for p in range(NP):
        SEL[2 * p, p * 128:p * 128 + 64] = 1.0
        SEL[2 * p + 1, p * 128 + 64:p * 128 + 128] = 1.0
    BS = np.zeros((128, L * NCH), np.float32)
    for l in range(L):
        for m in range(NCH):
            BS[:, l * NCH + m] = bias[l, m * 128:(m + 1) * 128]

    ON8 = np.ones((E, 1), np.float32)
    ON1 = np.ones((1, E), np.float32)
    shared = {"VT": VT, "UT": UT, "CB": CB, "GT": GT, "SEL": SEL, "BS": BS,
              "ON8": ON8, "ON1": ON1}
    if mmdt == "bf16":
        import ml_dtypes
        for k in ("VT", "UT", "CB", "GT", "SEL", "ON8", "ON1"):
            shared[k] = shared[k].astype(ml_dtypes.bfloat16)
        xT = xT.astype(ml_dtypes.bfloat16)
    in_maps = []
    for i in range(NCORES):
        m = dict(shared)
        m["xT"] = np.ascontiguousarray(xT[:, i * BC:(i + 1) * BC])
        in_maps.append(m)
    return in_maps


def run(nc, in_maps):
    res = run_bass_kernel_spmd(nc, in_maps, core_ids=list(range(NCORES)))
    yT = np.empty((N, B), np.float32)
    for i in range(NCORES):
        yT[:, i * BC:(i + 1) * BC] = res.results[i]["y"]
    return np.ascontiguousarray(yT.T)


_NC_CACHE = {}


def kernel(x, U, V, C, bias, gate_w):
    x = np.asarray(x)
    assert x.shape == (B, N), f"expected x {(B, N)}, got {x.shape}"
    if "nc" not in _NC_CACHE:
        _NC_CACHE["nc"] = build(niter=1)
    in_maps = pack_inputs(x, U, V, C, bias, gate_w)
    return run(_NC_CACHE["nc"], in_maps)
